# revision 8
# baseline (speedup 1.0000x reference)
"""BitLinear158 Trainium2 kernel (per-core body + host driver).

Per core: x_shard [M_LOC, K] bf16 -> per-token int8 quant -> bf16 matmul
against host-unpacked ternary wT [K, N] -> rescale -> y [M_LOC, N] bf16.

v3 pipeline (transposed quant, no DRAM round-trip):
  per block of 512 tokens:
    xT[kc]  [128,512] <- sync-ring DMA *transpose* straight from x DRAM
    amaxT   = DVE abs_max tree over the 16 k-chunks  [128,512]
    amaxR   = gpsimd partition_all_reduce(absmax)    (replicated on 128 parts)
    sT      = bf16(127 / amaxR)  (DVE, replicated)   [128,512]
    s_nat   <- tiny SBUF->SBUF DMA transposes of sT slices -> r_nat = 1/s (f32)
    t       = bf16(xT * sT)      (one DVE mult over all 16 kc; matches jax
                                  bf16 mul: f32 product, single bf16 round)
    xq8     = int8(t)            (DVE copy: RNE + saturate == round + clip)
    xqbf    = bf16(xq8)          (DVE copy)
  matmul:  PSUM [128m,512n] f32 += xqbf[kc][:,mi].T @ wt[:,kc,nt]  (16 kc)
  rescale: y_sb = ACT activation(Copy, scale=r_nat) -> bf16 ; y <- SWDGE DMA
  wt is loaded in 64 [128,512] chunks, nt-major, so the PE can start while
  the bulk of the weight is still in flight.
"""

import sys

sys.path.insert(0, "/opt/trn_rl_repo")

from contextlib import ExitStack

import numpy as np
import ml_dtypes

import concourse.bass as bass
import concourse.tile as tile
from concourse import bacc, mybir, bass_isa
from concourse import bass_utils

P = 128
M_LOC = 4096      # tokens per core
K = 2048          # in features
N = 2048          # out features
KC = K // P       # 16 k-chunks
BT = 256          # tokens per block
MB = M_LOC // BT  # 8 blocks
MPB = BT // P     # 4 m-tiles per block
NT = M_LOC // P   # 32 m-tiles per core
N_TILE = 512
NTN = N // N_TILE # 4
N_CORES = 8

BF16 = mybir.dt.bfloat16
F32 = mybir.dt.float32
I8 = mybir.dt.int8
I16 = mybir.dt.int16


def build_kernel():
    nc = bacc.Bacc("TRN2", target_bir_lowering=False, debug=False, num_devices=N_CORES)
    x = nc.dram_tensor("x", [M_LOC, K], BF16, kind="ExternalInput").ap()
    wT = nc.dram_tensor("wT", [K, N], BF16, kind="ExternalInput").ap()
    y = nc.dram_tensor("y", [M_LOC, N], BF16, kind="ExternalOutput").ap()

    y_tiled = y.rearrange("(t p) n -> t p n", p=P)
    wT_tiled = wT.rearrange("(c p) n -> p c n", p=P)

    with tile.TileContext(nc) as tc, ExitStack() as ctx:
        wbuf = ctx.enter_context(tc.tile_pool(name="wbuf", bufs=1))
        xtp = ctx.enter_context(tc.tile_pool(name="xtp", bufs=3))
        qtp = ctx.enter_context(tc.tile_pool(name="qtp", bufs=3))
        ttp = ctx.enter_context(tc.tile_pool(name="ttp", bufs=1))
        tree = ctx.enter_context(tc.tile_pool(name="tree", bufs=1))
        stat = ctx.enter_context(tc.tile_pool(name="stat", bufs=2))
        snat = ctx.enter_context(tc.tile_pool(name="snat", bufs=4))
        rbuf = ctx.enter_context(tc.tile_pool(name="rbuf", bufs=1))
        yout = ctx.enter_context(tc.tile_pool(name="yout", bufs=4))
        psum = ctx.enter_context(tc.tile_pool(name="psum", bufs=8, space="PSUM"))

        # weight: [128, kc, n]; 64 chunk DMAs, nt-major so the first MMs'
        # operands land first.
        wt = wbuf.tile([P, KC, N], BF16)
        for nt in range(NTN):
            for kc in range(KC):
                ns = slice(nt * N_TILE, (nt + 1) * N_TILE)
                nc.scalar.dma_start(wt[:, kc, ns], wT_tiled[:, kc, ns])

        r_all = rbuf.tile([P, NT], F32)

        def quant_block(b):
            xT = xtp.tile([P, KC, BT], BF16, tag="xT", name="xT")
            for kc in range(KC):
                nc.sync.dma_start_transpose(
                    xT[:, kc, :],
                    x[b * BT : (b + 1) * BT, kc * P : (kc + 1) * P],
                )
            # |x| by clearing the bf16 sign bit on an int16 view; for
            # non-negative IEEE values the int16 order matches the value
            # order, so the max tree can run in the int16 domain.
            absx = tree.tile([P, KC, BT], I16, tag="absx", name="absx")
            nc.vector.tensor_scalar(
                absx[:], xT[:].bitcast(I16), 0x7FFF, None,
                op0=mybir.AluOpType.bitwise_and,
            )
            tr1 = tree.tile([P, 8, BT], I16, tag="tr1", name="tr1")
            nc.vector.tensor_tensor(
                tr1[:], absx[:, 0::2, :], absx[:, 1::2, :], mybir.AluOpType.max
            )
            tr2 = tree.tile([P, 4, BT], I16, tag="tr2", name="tr2")
            nc.vector.tensor_tensor(
                tr2[:], tr1[:, 0::2, :], tr1[:, 1::2, :], mybir.AluOpType.max
            )
            tr3 = tree.tile([P, 2, BT], I16, tag="tr3", name="tr3")
            nc.vector.tensor_tensor(
                tr3[:], tr2[:, 0::2, :], tr2[:, 1::2, :], mybir.AluOpType.max
            )
            amax_bf = tree.tile([P, BT], I16, tag="amax_bf", name="amax_bf")
            nc.vector.tensor_tensor(
                amax_bf[:], tr3[:, 0, :], tr3[:, 1, :], mybir.AluOpType.max
            )
            amax_f = stat.tile([P, BT], F32, tag="amax_f", name="amax_f")
            nc.vector.tensor_copy(amax_f[:], amax_bf[:].bitcast(BF16))
            # replicate the per-token max across all 128 partitions
            amax_r = stat.tile([P, BT], F32, tag="amax_r", name="amax_r")
            nc.gpsimd.partition_all_reduce(
                amax_r[:], amax_f[:], channels=P, reduce_op=bass_isa.ReduceOp.absmax
            )
            nc.vector.tensor_scalar_max(amax_r[:], amax_r[:], 1e-5)
            q = stat.tile([P, BT], F32, tag="q", name="q")
            nc.vector.reciprocal(q[:], amax_r[:])
            sT = stat.tile([P, BT], BF16, tag="sT", name="sT")
            nc.vector.tensor_scalar_mul(sT[:], q[:], 127.0)

            # per-m-tile natural-layout scales for the output rescale
            for c in range(MPB):
                mt = b * MPB + c
                st = snat.tile([P, P], BF16, tag="st", name="st")
                nc.sync.dma_start_transpose(st[:], sT[:, c * P : (c + 1) * P])
                s32 = snat.tile([P, 1], F32, tag="s32", name="s32")
                nc.vector.tensor_copy(s32[:], st[:, 0:1])
                nc.vector.reciprocal(r_all[:, mt : mt + 1], s32[:])

            # quantize: bf16 product -> int8 (RNE+sat) -> bf16
            t = ttp.tile([P, KC, BT], BF16, tag="t", name="t")
            nc.vector.tensor_tensor(
                t[:], xT[:], sT[:, None, :].to_broadcast([P, KC, BT]),
                mybir.AluOpType.mult,
            )
            xq8 = ttp.tile([P, KC, BT], I8, tag="xq8", name="xq8")
            nc.vector.tensor_copy(xq8[:], t[:])
            xqbf = qtp.tile([P, KC, BT], BF16, tag="xqbf", name="xqbf")
            nc.vector.tensor_copy(xqbf[:], xq8[:])
            return xqbf

        def mm_block(b, xqbf):
            for mi in range(MPB):
                mt = b * MPB + mi
                y_sb = yout.tile([P, N], BF16, tag="y_sb", name="y_sb")
                for nt in range(NTN):
                    ps = psum.tile([P, N_TILE], F32, tag="ps", name="ps")
                    for kc in range(KC):
                        nc.tensor.matmul(
                            ps[:],
                            xqbf[:, kc, mi * P : (mi + 1) * P],
                            wt[:, kc, nt * N_TILE : (nt + 1) * N_TILE],
                            start=(kc == 0),
                            stop=(kc == KC - 1),
                        )
                    nc.scalar.activation(
                        y_sb[:, nt * N_TILE : (nt + 1) * N_TILE],
                        ps[:],
                        mybir.ActivationFunctionType.Copy,
                        scale=r_all[:, mt : mt + 1],
                    )
                nc.gpsimd.dma_start(y_tiled[mt], y_sb[:])

        xq_map = {0: quant_block(0), 1: quant_block(1)}
        for b in range(MB):
            if b + 2 < MB:
                xq_map[b + 2] = quant_block(b + 2)
            mm_block(b, xq_map.pop(b))

    nc.compile()
    return nc


def unpack_wT(packed_weight: np.ndarray, weight_scale: np.ndarray) -> np.ndarray:
    planes = [((packed_weight >> (2 * i)) & 3) for i in range(4)]
    w = np.concatenate(planes, 0).astype(np.float32) - 1.0  # [N, K]
    ws = np.float32(weight_scale.reshape(-1)[0])
    wT = np.ascontiguousarray((w / ws).T).astype(ml_dtypes.bfloat16)  # [K, N]
    return wT


_CACHE = {}


def run(x: np.ndarray, packed_weight: np.ndarray, weight_scale: np.ndarray,
        trace: bool = False, tmpdir=None):
    """x: [B, S, K] bf16 -> y [B, S, N] bf16 (full, unsharded)."""
    if "nc" not in _CACHE:
        _CACHE["nc"] = build_kernel()
    nc = _CACHE["nc"]

    B, S, D = x.shape
    M = B * S
    assert M == M_LOC * N_CORES and D == K
    wT = unpack_wT(packed_weight, weight_scale)
    shards = np.ascontiguousarray(np.asarray(x).reshape(N_CORES, M_LOC, K))
    in_maps = [{"x": shards[i], "wT": wT} for i in range(N_CORES)]
    res = bass_utils.run_bass_kernel_spmd(
        nc, in_maps, core_ids=list(range(N_CORES)), trace=trace, tmpdir=tmpdir
    )
    y = np.stack([res.results[i]["y"] for i in range(N_CORES)], axis=0)
    return y.reshape(B, S, N), res


def kernel(x, packed_weight, weight_scale):
    """Harness entrypoint: FULL inputs -> FULL output.

    x: [4, 8192, 2048] bf16; packed_weight: [512, 2048] uint8;
    weight_scale: [1] bf16.  Returns [4, 8192, 2048] bf16.
    Sharding: data-parallel over tokens across the 8 NeuronCores;
    the (host-unpacked) ternary weight is replicated.
    """
    x = np.asarray(x)
    packed_weight = np.asarray(packed_weight)
    weight_scale = np.asarray(weight_scale)
    y, _ = run(x, packed_weight, weight_scale)
    return y


# revision 12
# speedup vs baseline: 1.6554x; 1.6554x over previous
"""BitLinear158 Trainium2 kernel (per-core body + host driver).

Per core: xT_shard [K, M_LOC] bf16 (host-pre-transposed) -> per-token int8
quant -> bf16 matmul against host-unpacked ternary wT [K, N] -> rescale ->
y [M_LOC, N] bf16.

v4 pipeline (host-transposed input; all quant in [k-part, token] layout):
  per block of 512 tokens:
    xT      [128,16,512] <- ONE 3D-AP DMA (1KB contiguous segments)
    amaxT   = DVE max tree over kc on |x| (sign-bit cleared int16 view)
    amaxR   = gpsimd partition_all_reduce(absmax)  (replicated on 128 parts)
    sT      = bf16(127 / amaxR)  (DVE, replicated) [128,512]
    s_nat   <- tiny SBUF->SBUF DMA transposes of sT slices -> r_nat = 1/s (f32)
    t       = bf16(xT * sT)  (one DVE mult; f32 product, single bf16 round —
                              matches jax bf16 multiply semantics)
    xq8     = int8(t)        (DVE copy: RNE + saturate == round + clip)
    xqbf    = bf16(xq8)      (DVE copy)
  matmul:  PSUM [128m,512n] f32 += xqbf[kc][:,mi].T @ wt[:,kc,nt] (16 kc)
  rescale: y_sb = ACT activation(Copy, scale=r_nat) -> bf16 ; y <- scalar DMA
  wt is loaded in 64 [128,512] chunks, nt-major, so the PE can start while
  the bulk of the weight is still in flight.
"""

import sys

sys.path.insert(0, "/opt/trn_rl_repo")

from contextlib import ExitStack

import numpy as np
import ml_dtypes

import concourse.bass as bass
import concourse.tile as tile
from concourse import bacc, mybir, bass_isa
from concourse import bass_utils

P = 128
M_LOC = 4096      # tokens per core
K = 2048          # in features
N = 2048          # out features
KC = K // P       # 16 k-chunks
BT = 512          # tokens per block
MB = M_LOC // BT  # 8 blocks
MPB = BT // P     # 4 m-tiles per block
NT = M_LOC // P   # 32 m-tiles per core
N_TILE = 512
NTN = N // N_TILE # 4
N_CORES = 8

BF16 = mybir.dt.bfloat16
F32 = mybir.dt.float32
I8 = mybir.dt.int8
I16 = mybir.dt.int16


def build_kernel():
    nc = bacc.Bacc("TRN2", target_bir_lowering=False, debug=False, num_devices=N_CORES)
    xT_d = nc.dram_tensor("xT", [K, M_LOC], BF16, kind="ExternalInput").ap()
    wT = nc.dram_tensor("wT", [K, N], BF16, kind="ExternalInput").ap()
    y = nc.dram_tensor("y", [M_LOC, N], BF16, kind="ExternalOutput").ap()

    y_tiled = y.rearrange("(t p) n -> t p n", p=P)
    wT_tiled = wT.rearrange("(c p) n -> p c n", p=P)
    # [k-part, kc, token] view of the transposed input
    xT_tiled = xT_d.rearrange("(c p) m -> p c m", p=P)

    with tile.TileContext(nc) as tc, ExitStack() as ctx:
        wbuf = ctx.enter_context(tc.tile_pool(name="wbuf", bufs=1))
        xtp = ctx.enter_context(tc.tile_pool(name="xtp", bufs=2))
        qtp = ctx.enter_context(tc.tile_pool(name="qtp", bufs=2))
        ttp = ctx.enter_context(tc.tile_pool(name="ttp", bufs=1))
        tree = ctx.enter_context(tc.tile_pool(name="tree", bufs=1))
        stat = ctx.enter_context(tc.tile_pool(name="stat", bufs=2))
        snat = ctx.enter_context(tc.tile_pool(name="snat", bufs=4))
        rbuf = ctx.enter_context(tc.tile_pool(name="rbuf", bufs=1))
        yout = ctx.enter_context(tc.tile_pool(name="yout", bufs=4))
        psum = ctx.enter_context(tc.tile_pool(name="psum", bufs=8, space="PSUM"))

        # weight: [128, kc, n]; 64 chunk DMAs, nt-major so the first MMs'
        # operands land first.
        wt = wbuf.tile([P, KC, N], BF16)
        for nt in range(NTN):
            for kc in range(KC):
                ns = slice(nt * N_TILE, (nt + 1) * N_TILE)
                nc.scalar.dma_start(wt[:, kc, ns], wT_tiled[:, kc, ns])

        r_all = rbuf.tile([P, NT], F32)

        def quant_block(b):
            ms = slice(b * BT, (b + 1) * BT)
            xT = xtp.tile([P, KC, BT], BF16, tag="xT", name="xT")
            nc.sync.dma_start(xT[:], xT_tiled[:, :, ms])
            # |x| by clearing the bf16 sign bit on an int16 view; for
            # non-negative IEEE values int16 order matches value order,
            # so the max tree runs in the int16 domain. The scratch tile is
            # reused (bitcast) later in the block as the bf16 product `t` —
            # the abs values are dead by then.
            scr = ttp.tile([P, KC, BT], I16, tag="scr", name="scr")
            absx = scr[:]
            nc.vector.tensor_scalar(
                absx, xT[:].bitcast(I16), 0x7FFF, None,
                op0=mybir.AluOpType.bitwise_and,
            )
            tr1 = tree.tile([P, 8, BT], I16, tag="tr1", name="tr1")
            nc.vector.tensor_tensor(
                tr1[:], absx[:, 0::2, :], absx[:, 1::2, :], mybir.AluOpType.max
            )
            del absx
            tr2 = tree.tile([P, 4, BT], I16, tag="tr2", name="tr2")
            nc.vector.tensor_tensor(
                tr2[:], tr1[:, 0::2, :], tr1[:, 1::2, :], mybir.AluOpType.max
            )
            tr3 = tree.tile([P, 2, BT], I16, tag="tr3", name="tr3")
            nc.vector.tensor_tensor(
                tr3[:], tr2[:, 0::2, :], tr2[:, 1::2, :], mybir.AluOpType.max
            )
            amax_bf = tree.tile([P, BT], I16, tag="amax_bf", name="amax_bf")
            nc.vector.tensor_tensor(
                amax_bf[:], tr3[:, 0, :], tr3[:, 1, :], mybir.AluOpType.max
            )
            amax_f = stat.tile([P, BT], F32, tag="amax_f", name="amax_f")
            nc.vector.tensor_copy(amax_f[:], amax_bf[:].bitcast(BF16))
            # replicate the per-token max across all 128 partitions
            amax_r = stat.tile([P, BT], F32, tag="amax_r", name="amax_r")
            nc.gpsimd.partition_all_reduce(
                amax_r[:], amax_f[:], channels=P, reduce_op=bass_isa.ReduceOp.absmax
            )
            nc.vector.tensor_scalar_max(amax_r[:], amax_r[:], 1e-5)
            q = stat.tile([P, BT], F32, tag="q", name="q")
            nc.vector.reciprocal(q[:], amax_r[:])
            sT = stat.tile([P, BT], BF16, tag="sT", name="sT")
            nc.vector.tensor_scalar_mul(sT[:], q[:], 127.0)

            # per-m-tile natural-layout scales for the output rescale
            for c in range(MPB):
                mt = b * MPB + c
                st = snat.tile([P, P], BF16, tag="st", name="st")
                nc.sync.dma_start_transpose(st[:], sT[:, c * P : (c + 1) * P])
                s32 = snat.tile([P, 1], F32, tag="s32", name="s32")
                nc.vector.tensor_copy(s32[:], st[:, 0:1])
                nc.vector.reciprocal(r_all[:, mt : mt + 1], s32[:])

            # quantize: bf16 product -> int8 (RNE+sat) -> bf16
            t = scr[:].bitcast(BF16)
            nc.vector.tensor_tensor(
                t, xT[:], sT[:, None, :].to_broadcast([P, KC, BT]),
                mybir.AluOpType.mult,
            )
            xq8 = ttp.tile([P, KC, BT], I8, tag="xq8", name="xq8")
            nc.vector.tensor_copy(xq8[:], t)
            xqbf = qtp.tile([P, KC, BT], BF16, tag="xqbf", name="xqbf")
            nc.vector.tensor_copy(xqbf[:], xq8[:])
            return xqbf

        def mm_block(b, xqbf):
            for mi in range(MPB):
                mt = b * MPB + mi
                y_sb = yout.tile([P, N], BF16, tag="y_sb", name="y_sb")
                for nt in range(NTN):
                    ps = psum.tile([P, N_TILE], F32, tag="ps", name="ps")
                    for kc in range(KC):
                        nc.tensor.matmul(
                            ps[:],
                            xqbf[:, kc, mi * P : (mi + 1) * P],
                            wt[:, kc, nt * N_TILE : (nt + 1) * N_TILE],
                            start=(kc == 0),
                            stop=(kc == KC - 1),
                        )
                    nc.scalar.activation(
                        y_sb[:, nt * N_TILE : (nt + 1) * N_TILE],
                        ps[:],
                        mybir.ActivationFunctionType.Copy,
                        scale=r_all[:, mt : mt + 1],
                    )
                nc.scalar.dma_start(y_tiled[mt], y_sb[:])

        xq_map = {0: quant_block(0)}
        for b in range(MB):
            if b + 1 < MB:
                xq_map[b + 1] = quant_block(b + 1)
            mm_block(b, xq_map.pop(b))

    nc.compile()
    return nc


def unpack_wT(packed_weight: np.ndarray, weight_scale: np.ndarray) -> np.ndarray:
    planes = [((packed_weight >> (2 * i)) & 3) for i in range(4)]
    w = np.concatenate(planes, 0).astype(np.float32) - 1.0  # [N, K]
    ws = np.float32(weight_scale.reshape(-1)[0])
    wT = np.ascontiguousarray((w / ws).T).astype(ml_dtypes.bfloat16)  # [K, N]
    return wT


_CACHE = {}


def run(x: np.ndarray, packed_weight: np.ndarray, weight_scale: np.ndarray,
        trace: bool = False, tmpdir=None):
    """x: [B, S, K] bf16 -> y [B, S, N] bf16 (full, unsharded)."""
    if "nc" not in _CACHE:
        _CACHE["nc"] = build_kernel()
    nc = _CACHE["nc"]

    B, S, D = x.shape
    M = B * S
    assert M == M_LOC * N_CORES and D == K
    wT = unpack_wT(packed_weight, weight_scale)
    shards = np.asarray(x).reshape(N_CORES, M_LOC, K)
    in_maps = [
        {"xT": np.ascontiguousarray(shards[i].T), "wT": wT}
        for i in range(N_CORES)
    ]
    res = bass_utils.run_bass_kernel_spmd(
        nc, in_maps, core_ids=list(range(N_CORES)), trace=trace, tmpdir=tmpdir
    )
    y = np.stack([res.results[i]["y"] for i in range(N_CORES)], axis=0)
    return y.reshape(B, S, N), res


def kernel(x, packed_weight, weight_scale):
    """Harness entrypoint: FULL inputs -> FULL output.

    x: [4, 8192, 2048] bf16; packed_weight: [512, 2048] uint8;
    weight_scale: [1] bf16.  Returns [4, 8192, 2048] bf16.
    Sharding: data-parallel over tokens across the 8 NeuronCores;
    the (host-unpacked) ternary weight is replicated.
    """
    x = np.asarray(x)
    packed_weight = np.asarray(packed_weight)
    weight_scale = np.asarray(weight_scale)
    y, _ = run(x, packed_weight, weight_scale)
    return y


# revision 14
# speedup vs baseline: 1.6796x; 1.0146x over previous
"""BitLinear158 Trainium2 kernel (per-core body + host driver).

Per core: xT_shard [K, M_LOC] bf16 (host-pre-transposed) -> per-token int8
quant -> bf16 matmul against host-unpacked ternary wT [K, N] -> rescale ->
y [M_LOC, N] bf16.

v4 pipeline (host-transposed input; all quant in [k-part, token] layout):
  per block of 512 tokens:
    xT      [128,16,512] <- ONE 3D-AP DMA (1KB contiguous segments)
    amaxT   = DVE max tree over kc on |x| (sign-bit cleared int16 view)
    amaxR   = gpsimd partition_all_reduce(absmax)  (replicated on 128 parts)
    sT      = bf16(127 / amaxR)  (DVE, replicated) [128,512]
    s_nat   <- tiny SBUF->SBUF DMA transposes of sT slices -> r_nat = 1/s (f32)
    t       = bf16(xT * sT)  (one DVE mult; f32 product, single bf16 round —
                              matches jax bf16 multiply semantics)
    xq8     = int8(t)        (DVE copy: RNE + saturate == round + clip)
    xqbf    = bf16(xq8)      (DVE copy)
  matmul:  PSUM [128m,512n] f32 += xqbf[kc][:,mi].T @ wt[:,kc,nt] (16 kc)
  rescale: y_sb = ACT activation(Copy, scale=r_nat) -> bf16 ; y <- scalar DMA
  wt is loaded in 64 [128,512] chunks, nt-major, so the PE can start while
  the bulk of the weight is still in flight.
"""

import sys

sys.path.insert(0, "/opt/trn_rl_repo")

from contextlib import ExitStack

import numpy as np
import ml_dtypes

import concourse.bass as bass
import concourse.tile as tile
from concourse import bacc, mybir, bass_isa
from concourse import bass_utils

P = 128
M_LOC = 4096      # tokens per core
K = 2048          # in features
N = 2048          # out features
KC = K // P       # 16 k-chunks
BT = 512          # max tokens per block (buffer sizing)
# small leading blocks so the PE starts early; small last block for the tail
BTS = [128, 256, 512, 512, 512, 512, 512, 512, 512, 128]
assert sum(BTS) == M_LOC and all(b % P == 0 for b in BTS)
BSTART = [sum(BTS[:i]) for i in range(len(BTS))]
MB = len(BTS)
NT = M_LOC // P   # 32 m-tiles per core
N_TILE = 512
NTN = N // N_TILE # 4
N_CORES = 8

BF16 = mybir.dt.bfloat16
F32 = mybir.dt.float32
I8 = mybir.dt.int8
I16 = mybir.dt.int16


def build_kernel():
    nc = bacc.Bacc("TRN2", target_bir_lowering=False, debug=False, num_devices=N_CORES)
    xT_d = nc.dram_tensor("xT", [K, M_LOC], BF16, kind="ExternalInput").ap()
    wT = nc.dram_tensor("wT", [K, N], BF16, kind="ExternalInput").ap()
    y = nc.dram_tensor("y", [M_LOC, N], BF16, kind="ExternalOutput").ap()

    y_tiled = y.rearrange("(t p) n -> t p n", p=P)
    wT_tiled = wT.rearrange("(c p) n -> p c n", p=P)
    # [k-part, kc, token] view of the transposed input
    xT_tiled = xT_d.rearrange("(c p) m -> p c m", p=P)

    with tile.TileContext(nc) as tc, ExitStack() as ctx:
        wbuf = ctx.enter_context(tc.tile_pool(name="wbuf", bufs=1))
        xtp = ctx.enter_context(tc.tile_pool(name="xtp", bufs=2))
        qtp = ctx.enter_context(tc.tile_pool(name="qtp", bufs=2))
        ttp = ctx.enter_context(tc.tile_pool(name="ttp", bufs=1))
        tree = ctx.enter_context(tc.tile_pool(name="tree", bufs=1))
        stat = ctx.enter_context(tc.tile_pool(name="stat", bufs=2))
        snat = ctx.enter_context(tc.tile_pool(name="snat", bufs=4))
        rbuf = ctx.enter_context(tc.tile_pool(name="rbuf", bufs=1))
        yout = ctx.enter_context(tc.tile_pool(name="yout", bufs=4))
        psum = ctx.enter_context(tc.tile_pool(name="psum", bufs=8, space="PSUM"))

        # weight: [128, kc, n]; 64 chunk DMAs, nt-major so the first MMs'
        # operands land first.
        wt = wbuf.tile([P, KC, N], BF16)
        for nt in range(NTN):
            for kc in range(KC):
                ns = slice(nt * N_TILE, (nt + 1) * N_TILE)
                nc.scalar.dma_start(wt[:, kc, ns], wT_tiled[:, kc, ns])

        r_all = rbuf.tile([P, NT], F32)

        def quant_block(b):
            bt = BTS[b]
            ms = slice(BSTART[b], BSTART[b] + bt)
            xT = xtp.tile([P, KC, BT], BF16, tag="xT", name="xT")[:, :, :bt]
            nc.sync.dma_start(xT, xT_tiled[:, :, ms])
            # |x| by clearing the bf16 sign bit on an int16 view; for
            # non-negative IEEE values int16 order matches value order,
            # so the max tree runs in the int16 domain. The scratch tile is
            # reused (bitcast) later in the block as the bf16 product `t` —
            # the abs values are dead by then.
            scr = ttp.tile([P, KC, BT], I16, tag="scr", name="scr")[:, :, :bt]
            nc.vector.tensor_scalar(
                scr, xT.bitcast(I16), 0x7FFF, None,
                op0=mybir.AluOpType.bitwise_and,
            )
            tr1 = tree.tile([P, 8, BT], I16, tag="tr1", name="tr1")[:, :, :bt]
            nc.vector.tensor_tensor(
                tr1, scr[:, 0::2, :], scr[:, 1::2, :], mybir.AluOpType.max
            )
            tr2 = tree.tile([P, 4, BT], I16, tag="tr2", name="tr2")[:, :, :bt]
            nc.vector.tensor_tensor(
                tr2, tr1[:, 0::2, :], tr1[:, 1::2, :], mybir.AluOpType.max
            )
            tr3 = tree.tile([P, 2, BT], I16, tag="tr3", name="tr3")[:, :, :bt]
            nc.vector.tensor_tensor(
                tr3, tr2[:, 0::2, :], tr2[:, 1::2, :], mybir.AluOpType.max
            )
            amax_bf = tree.tile([P, BT], I16, tag="amax_bf", name="amax_bf")[:, :bt]
            nc.vector.tensor_tensor(
                amax_bf, tr3[:, 0, :], tr3[:, 1, :], mybir.AluOpType.max
            )
            amax_f = stat.tile([P, BT], F32, tag="amax_f", name="amax_f")[:, :bt]
            nc.vector.tensor_copy(amax_f, amax_bf.bitcast(BF16))
            # replicate the per-token max across all 128 partitions
            amax_r = stat.tile([P, BT], F32, tag="amax_r", name="amax_r")[:, :bt]
            nc.gpsimd.partition_all_reduce(
                amax_r, amax_f, channels=P, reduce_op=bass_isa.ReduceOp.absmax
            )
            nc.vector.tensor_scalar_max(amax_r, amax_r, 1e-5)
            q = stat.tile([P, BT], F32, tag="q", name="q")[:, :bt]
            nc.vector.reciprocal(q, amax_r)
            sT = stat.tile([P, BT], BF16, tag="sT", name="sT")[:, :bt]
            nc.vector.tensor_scalar_mul(sT, q, 127.0)

            # quantize: bf16 product -> int8 (RNE+sat) -> bf16
            # (emitted ahead of the r_nat chain: the s32 copies below wait on
            # DMA transposes and must not head-of-line-block these big casts)
            t = scr.bitcast(BF16)
            nc.vector.tensor_tensor(
                t, xT, sT[:, None, :].to_broadcast([P, KC, bt]),
                mybir.AluOpType.mult,
            )
            xq8 = ttp.tile([P, KC, BT], I8, tag="xq8", name="xq8")[:, :, :bt]
            nc.vector.tensor_copy(xq8, t)
            xqbf = qtp.tile([P, KC, BT], BF16, tag="xqbf", name="xqbf")[:, :, :bt]
            nc.vector.tensor_copy(xqbf, xq8)

            # per-m-tile natural-layout scales for the output rescale
            for c in range(bt // P):
                mt = BSTART[b] // P + c
                st = snat.tile([P, P], BF16, tag="st", name="st")
                nc.sync.dma_start_transpose(st[:], sT[:, c * P : (c + 1) * P])
                s32 = snat.tile([P, 1], F32, tag="s32", name="s32")
                nc.vector.tensor_copy(s32[:], st[:, 0:1])
                nc.vector.reciprocal(r_all[:, mt : mt + 1], s32[:])
            return xqbf

        def mm_block(b, xqbf):
            for mi in range(BTS[b] // P):
                mt = BSTART[b] // P + mi
                y_sb = yout.tile([P, N], BF16, tag="y_sb", name="y_sb")
                for nt in range(NTN):
                    ps = psum.tile([P, N_TILE], F32, tag="ps", name="ps")
                    for kc in range(KC):
                        nc.tensor.matmul(
                            ps[:],
                            xqbf[:, kc, mi * P : (mi + 1) * P],
                            wt[:, kc, nt * N_TILE : (nt + 1) * N_TILE],
                            start=(kc == 0),
                            stop=(kc == KC - 1),
                        )
                    nc.scalar.activation(
                        y_sb[:, nt * N_TILE : (nt + 1) * N_TILE],
                        ps[:],
                        mybir.ActivationFunctionType.Copy,
                        scale=r_all[:, mt : mt + 1],
                    )
                nc.sync.dma_start(y_tiled[mt], y_sb[:])

        xq_map = {0: quant_block(0)}
        for b in range(MB):
            if b + 1 < MB:
                xq_map[b + 1] = quant_block(b + 1)
            mm_block(b, xq_map.pop(b))

    nc.compile()
    return nc


def unpack_wT(packed_weight: np.ndarray, weight_scale: np.ndarray) -> np.ndarray:
    planes = [((packed_weight >> (2 * i)) & 3) for i in range(4)]
    w = np.concatenate(planes, 0).astype(np.float32) - 1.0  # [N, K]
    ws = np.float32(weight_scale.reshape(-1)[0])
    wT = np.ascontiguousarray((w / ws).T).astype(ml_dtypes.bfloat16)  # [K, N]
    return wT


_CACHE = {}


def run(x: np.ndarray, packed_weight: np.ndarray, weight_scale: np.ndarray,
        trace: bool = False, tmpdir=None):
    """x: [B, S, K] bf16 -> y [B, S, N] bf16 (full, unsharded)."""
    if "nc" not in _CACHE:
        _CACHE["nc"] = build_kernel()
    nc = _CACHE["nc"]

    B, S, D = x.shape
    M = B * S
    assert M == M_LOC * N_CORES and D == K
    wT = unpack_wT(packed_weight, weight_scale)
    shards = np.asarray(x).reshape(N_CORES, M_LOC, K)
    in_maps = [
        {"xT": np.ascontiguousarray(shards[i].T), "wT": wT}
        for i in range(N_CORES)
    ]
    res = bass_utils.run_bass_kernel_spmd(
        nc, in_maps, core_ids=list(range(N_CORES)), trace=trace, tmpdir=tmpdir
    )
    y = np.stack([res.results[i]["y"] for i in range(N_CORES)], axis=0)
    return y.reshape(B, S, N), res


def kernel(x, packed_weight, weight_scale):
    """Harness entrypoint: FULL inputs -> FULL output.

    x: [4, 8192, 2048] bf16; packed_weight: [512, 2048] uint8;
    weight_scale: [1] bf16.  Returns [4, 8192, 2048] bf16.
    Sharding: data-parallel over tokens across the 8 NeuronCores;
    the (host-unpacked) ternary weight is replicated.
    """
    x = np.asarray(x)
    packed_weight = np.asarray(packed_weight)
    weight_scale = np.asarray(weight_scale)
    y, _ = run(x, packed_weight, weight_scale)
    return y
